# revision 36
# baseline (speedup 1.0000x reference)
"""CCNF RK4 sampling kernel for 8 Trainium2 NeuronCores.

Data-parallel: batch 2048 -> 256 per core, weights replicated.
On-core layout: features on partitions, batch on the free dim (N=256).
All network matmuls in bf16 (1 cyc/row on the PE; rel err ~9e-4 vs the
2e-2 gate).

Over the v1 baseline (463066 -> 450139 ns, cost model):
  - t-row folded into a host-precomputed per-t bias table (33 distinct
    t values), removing the per-eval memset from the serial RK4 chain.
  - bf16 weights/activations halve the weight DMA footprint.
  - startup DMA chain minimized: HWDGE prep is 625ns/DMA (serialized)
    and completion-sem propagation 900ns, so the L1-critical tensors
    ship as two packed DMAs ([ctx | W1-ctx-b | W1-ctx-a] and
    [theta0 | W1-theta]), w2 streams per k-chunk, bias tables ride one
    DMA, and the unused ones/b3 constant is skipped when b3 == 0.
  - L3 shares the 8-bank PSUM ring (no dedicated bank).

Per-eval steady state (cost model): 52 matmuls = 5.55us PE busy plus
~1.3us exposed serial-chain latency (theta-arg STT -> L1 -> sigmoid ->
GLU fill, plus GLU->L3->STT tail) that the 0.85us of chain-independent
ctx-matmul filler cannot fully cover. Measured dead ends (all worse):
128-column stream splits (per-op ACT/DVE overheads ~200-250ns exceed
the latency hidden), kc-rotation in L2 groups (removes scheduler
freedom), PSUM pre-fill / filler shuffling (wall time is invariant:
work + exposed chain is conserved under any reordering the scheduler
can already do).
"""

import os

import numpy as np
from ml_dtypes import bfloat16 as _bf16
from ml_dtypes import float8_e4m3 as _f8np

N_CORES = 8


def _build_program(theta0, context, W1, b1, W2, b2, W3, b3, n_steps):
    import concourse.bass as bass
    import concourse.mybir as mybir
    import concourse.tile as tile
    from concourse import bacc

    f32 = mybir.dt.float32
    f32r = mybir.dt.float32r
    bf16 = mybir.dt.bfloat16
    f8 = mybir.dt.float8e4
    DR = mybir.MatmulPerfMode.DoubleRow
    ALU = mybir.AluOpType
    SIGMOID = mybir.ActivationFunctionType.Sigmoid

    B, D = theta0.shape          # 2048, 32
    C = context.shape[1]         # 128
    IN, H2 = W1.shape            # 161, 1024
    H = W2.shape[0]              # 512
    assert H2 == 2 * H and W2.shape[1] == 2 * H and W3.shape == (H, D)
    assert IN == D + 1 + C
    assert B % N_CORES == 0
    Bs = B // N_CORES            # 256 per core
    steps = int(n_steps)
    dt = 1.0 / steps

    KC = H // 128                # 4 k-chunks for layer 2/3
    MJ = H // 128                # 4 output column-chunks per GLU half
    K1B = C                      # 128 ctx rows

    # ---- host-side layout prep (shared across cores) ----
    W1 = np.asarray(W1, np.float32)
    w1th_h = np.ascontiguousarray(W1[:D])                      # [32, 1024] theta rows
    w1t_row = W1[D]                                            # [1024] time row
    w1c2_h = np.ascontiguousarray(W1[D + 1 :])                 # [128, 1024] ctx rows
    b1 = np.asarray(b1, np.float32)
    b2 = np.asarray(b2, np.float32)
    b3_is_zero = not np.any(np.asarray(b3, np.float32))
    # DoubleRow layout: [512,1024] -> [128, pair(2), plane(2), 1024]
    # (pair P covers kc = 2P, 2P+1; plane i is the kc = 2P+i k-tile)
    KCP = KC // 2
    w2_h = np.ascontiguousarray(
        np.asarray(W2, np.float32)
        .reshape(KCP, 2, 128, 2 * H).transpose(2, 0, 1, 3)
        .reshape(128, KCP * 2 * 2 * H)
    )
    # [512,32] -> [128, 4*32]
    w3_h = np.ascontiguousarray(
        np.asarray(W3, np.float32).reshape(KC, 128, D).transpose(1, 0, 2).reshape(128, KC * D)
    )
    # per-t layer-1 bias table: bias(t) = b1 + t * w1t_row, t = k*dt/2 for
    # k = 0..2*steps. layout per t: [a-half j cols | b-half j cols] = [128, 8]
    NT = 2 * steps + 1
    tvals = (np.arange(NT, dtype=np.float32) * (dt / 2.0)).reshape(NT, 1)
    btab = b1.reshape(1, 2 * H) + tvals * w1t_row.reshape(1, 2 * H)  # [NT, 1024]
    btab = btab.reshape(NT, 2, MJ, 128)                              # (t, half, j, p)
    bias_t_h = np.ascontiguousarray(
        btab.transpose(3, 0, 1, 2).reshape(128, NT * 2 * MJ)
    )                                                          # [128, NT*8]
    bias2_h = np.concatenate([
        b2[:H].reshape(MJ, 128).T, b2[H:].reshape(MJ, 128).T,
    ], axis=1)                                                 # [128, 8]
    bias_t_h = np.ascontiguousarray(
        np.concatenate([bias_t_h, bias2_h], axis=1)
    )                                                          # [128, NT*8+8]
    ctxpack_h = None  # built per-core (contains the ctx shard)
    onesb3_h = np.ascontiguousarray(np.concatenate([
        np.ones((1, Bs), np.float32),
        np.asarray(b3, np.float32).reshape(1, D),
    ], axis=1))                                                # [1, Bs+32]
    # onesb3 only ships when b3 is nonzero (it is zero for this problem)

    # ---- build the bass program (same program on all 8 cores) ----
    nc = bacc.Bacc("TRN2", target_bir_lowering=False)

    d_th0 = nc.dram_tensor("th0", [D, Bs], f32r, kind="ExternalInput")
    # packed: [x2 (Bs) | w1c2 b-half (H) | w1c2 a-half (H)] — one partition
    # group, two DMAs (critical prefix first), one HWDGE prep each
    d_ctxpack = nc.dram_tensor("ctxpack", [K1B, Bs + 2 * H], bf16,
                               kind="ExternalInput")
    # packed: [th0h (Bs) | w1th (2H)]
    d_thpack = nc.dram_tensor("thpack", [D, Bs + 2 * H], bf16,
                              kind="ExternalInput")
    d_w2 = nc.dram_tensor("w2", [128, KCP * 2 * 2 * H], f8, kind="ExternalInput")
    d_w3 = nc.dram_tensor("w3", [128, KC * D], bf16, kind="ExternalInput")
    d_biast = nc.dram_tensor("biast", [128, NT * 2 * MJ + 2 * MJ], f32, kind="ExternalInput")
    d_ob3 = (None if b3_is_zero else
             nc.dram_tensor("onesb3", [1, Bs + D], f32r, kind="ExternalInput"))
    d_out = nc.dram_tensor("out", [D, Bs], f32, kind="ExternalOutput")

    # RK4 coefficients: arg scale (for next eval's input), acc scale
    c_arg = [0.5 * dt, 0.5 * dt, dt]
    a_acc = [dt / 6.0, dt / 3.0, dt / 3.0, dt / 6.0]
    # t index per (step, eval): t = (s + TOFF[e]) * dt -> idx = 2s + IOFF[e]
    IOFF = (0, 1, 1, 2)

    FSPLIT = int(os.environ.get("KERNEL_FSPLIT", "2"))  # ctx banks in stall-1 slot

    with tile.TileContext(nc) as tc:
        with (
            tc.tile_pool(name="const", bufs=1) as cpool,
            tc.tile_pool(name="psmm", bufs=8, space="PSUM") as ps_pool,
            tc.tile_pool(name="sig", bufs=int(os.environ.get("KERNEL_SIGB", "10"))) as sig_pool,
            tc.tile_pool(name="hact", bufs=int(os.environ.get("KERNEL_HB", "20"))) as h_pool,
            tc.tile_pool(name="accp", bufs=int(os.environ.get("KERNEL_AB", "6"))) as acc_pool,
        ):
            tctx = cpool.tile([K1B, Bs + 2 * H], bf16)
            tx2 = tctx[:, 0:Bs]
            # w1c2 columns: b-half at [Bs : Bs+H], a-half at [Bs+H : Bs+2H]
            tthp = cpool.tile([D, Bs + 2 * H], bf16)
            tx1 = tthp[:, 0:Bs]
            tw1th = tthp[:, Bs : Bs + 2 * H]
            tw2 = cpool.tile([128, KCP, 2, 2 * H], f8)
            tw3 = cpool.tile([128, KC * D], bf16)
            tbiast = cpool.tile([128, NT * 2 * MJ + 2 * MJ], f32)
            tb2a = tbiast[:, NT * 2 * MJ : NT * 2 * MJ + MJ]
            tb2b = tbiast[:, NT * 2 * MJ + MJ : NT * 2 * MJ + 2 * MJ]
            if not b3_is_zero:
                tob3 = cpool.tile([1, Bs + D], f32r)
                tones = tob3[:, 0:Bs]
                tb3 = tob3[:, Bs : Bs + D]
            tth0 = cpool.tile([D, Bs], f32r)    # initial theta

            def w1c2_col(mj):
                # mj >= MJ: b-half chunk, else a-half chunk
                if mj >= MJ:
                    base = Bs + (mj - MJ) * 128
                else:
                    base = Bs + H + mj * 128
                return tctx[:, base : base + 128]

            def tb1a(idx, j):
                return tbiast[:, idx * 2 * MJ + j : idx * 2 * MJ + j + 1]

            def tb1b(idx, j):
                return tbiast[:, idx * 2 * MJ + MJ + j : idx * 2 * MJ + MJ + j + 1]

            # L1-critical tensors first so eval 0 can start while w2/w3
            # still stream; w2 split per k-chunk so layer 2 can begin
            # before the full weight matrix lands.
            nc.sync.dma_start(tctx[:, 0 : Bs + H], d_ctxpack[:, 0 : Bs + H])
            nc.sync.dma_start(tctx[:, Bs + H : Bs + 2 * H],
                              d_ctxpack[:, Bs + H : Bs + 2 * H])
            nc.sync.dma_start(tthp[:], d_thpack[:])
            nc.sync.dma_start(tbiast[:], d_biast[:])
            for P in range(KCP):
                nc.sync.dma_start(
                    tw2[:, P, :, :],
                    d_w2[:, P * 2 * 2 * H : (P + 1) * 2 * 2 * H],
                )
                if P == 0:
                    nc.sync.dma_start(tth0[:], d_th0[:])
            nc.sync.dma_start(tw3[:], d_w3[:])
            if not b3_is_zero:
                nc.sync.dma_start(tob3[:], d_ob3[:])

            def mm(out_ap, lhsT_ap, rhs_ap, start, stop, pm=None):
                nc.tensor.matmul(out_ap, lhsT_ap, rhs_ap, start=start,
                                 stop=stop, perf_mode=pm)

            th_cur = tth0       # theta at start of current step

            def issue_l1ctx(js):
                # static context contribution for the NEXT eval's layer 1,
                # placed in PE stall windows. One accumulation group per
                # bank; stop goes on the last theta MM next eval.
                tiles = []
                for j in js:
                    ps = ps_pool.tile([128, 2 * Bs], f32, tag="psmm")
                    for half, mj in ((1, j + MJ), (0, j)):
                        dst = ps[:, half * Bs : (half + 1) * Bs]
                        mm(dst, w1c2_col(mj), tx2[:],
                           start=(half == 1), stop=False)
                    tiles.append(ps)
                return tiles

            # only 3 of 4 L1 banks are pre-issued: 8 bank allocs per eval on
            # the 8-slot ring (a 4th pre-issue would shift slot reuse into
            # live banks and stall ~650ns/eval on bank-free waits). Bank j=3
            # gets its ctx MMs inline, right before its theta MMs.
            ps1 = issue_l1ctx(range(MJ - 1))

            for s in range(steps):
                for e in range(4):
                    idx = 2 * s + IOFF[e]
                    last_eval = (s == steps - 1) and (e == 3)

                    # ---- layer 1: theta MMs close the pre-issued banks ----
                    # h1 chunks land in fp8 pair tiles [128, plane(2), Bs]
                    # (plane = kc within the pair) feeding DoubleRow L2 MMs
                    h1p = [h_pool.tile([128, 2, Bs], f8, tag="h1t",
                                       name=f"h1p{P}")
                           for P in range(KCP)]
                    for j in range(MJ):
                        if j < len(ps1):
                            ps = ps1[j]
                            first = False
                        else:
                            ps = ps_pool.tile([128, 2 * Bs], f32, tag="psmm")
                            first = True
                        for half, mj in ((1, j + MJ), (0, j)):
                            dst = ps[:, half * Bs : (half + 1) * Bs]
                            if first:
                                # start=True only on the bank's first MM: a
                                # second start would re-mark the whole bank
                                # pending-zero and wipe the b-half sums
                                mm(dst, w1c2_col(mj), tx2[:],
                                   start=(half == 1), stop=False)
                        for half, mj in ((1, j + MJ), (0, j)):
                            dst = ps[:, half * Bs : (half + 1) * Bs]
                            msl = slice(mj * 128, (mj + 1) * 128)
                            mm(dst, tw1th[:, msl], tx1[:], start=False,
                               stop=(half == 0))
                        sg = sig_pool.tile([128, Bs], f32, tag="sig1")
                        nc.scalar.activation(
                            sg[:], ps[:, Bs : 2 * Bs], SIGMOID,
                            bias=tb1b(idx, j)
                        )
                        nc.vector.scalar_tensor_tensor(
                            h1p[j // 2][:, j % 2, :], ps[:, 0:Bs],
                            tb1a(idx, j), sg[:],
                            ALU.add, ALU.mult,
                        )

                    # first chunk of next-eval ctx MMs fills the h1[0] wait
                    if not last_eval:
                        ps1_next = issue_l1ctx(range(FSPLIT))

                    # ---- layer 2 ----
                    h2 = []
                    for j in range(MJ):
                        ps = ps_pool.tile([128, 2 * Bs], f32, tag="psmm")
                        # b-half group first so the sigmoid overlaps the
                        # a-half matmuls
                        dstb = ps[:, Bs : 2 * Bs]
                        for P in range(KCP):
                            csl = slice((j + MJ) * 128, (j + MJ + 1) * 128)
                            mm(dstb, tw2[:, P, :, csl], h1p[P][:],
                               start=(P == 0), stop=(P == KCP - 1), pm=DR)
                        sg = sig_pool.tile([128, Bs], f32, tag="sig2")
                        nc.scalar.activation(
                            sg[:], dstb, SIGMOID, bias=tb2b[:, j : j + 1]
                        )
                        dsta = ps[:, 0:Bs]
                        for P in range(KCP):
                            csl = slice(j * 128, (j + 1) * 128)
                            mm(dsta, tw2[:, P, :, csl], h1p[P][:],
                               start=(P == 0), stop=(P == KCP - 1), pm=DR)
                        ht = h_pool.tile([128, Bs], bf16, tag="h2t")
                        nc.vector.scalar_tensor_tensor(
                            ht[:], dsta, tb2a[:, j : j + 1], sg[:],
                            ALU.add, ALU.mult,
                        )
                        h2.append(ht)

                    # ---- layer 3: k = h2 @ W3 (+ b3) in PSUM ----
                    ps3full = ps_pool.tile([128, 2 * Bs], f32, tag="psmm")
                    ps3 = ps3full[0:D, 0:Bs]
                    for kc in range(KC):
                        mm(ps3[:], tw3[:, kc * D : (kc + 1) * D], h2[kc][:],
                           start=(kc == 0), stop=(kc == KC - 1 and b3_is_zero))
                    if not b3_is_zero:
                        mm(ps3[:], tb3[:], tones[:], start=False, stop=True)

                    # remaining pre-issued ctx MMs fill the tx1 wait at the
                    # boundary (bank j=3 is issued inline next eval)
                    if not last_eval:
                        ps1_next += issue_l1ctx(range(FSPLIT, MJ - 1))

                    # ---- RK4 bookkeeping ----
                    base = th_cur if e == 0 else acc_prev
                    if e < 3:
                        # next eval's theta arg (critical: feeds L1)
                        nc.vector.scalar_tensor_tensor(
                            tx1[:], ps3[:], float(c_arg[e]), th_cur[:],
                            ALU.mult, ALU.add,
                        )
                    elif s != steps - 1:
                        # theta_{s+1} straight into the matmul input tile
                        nc.vector.scalar_tensor_tensor(
                            tx1[:], ps3[:], float(a_acc[e]), base[:],
                            ALU.mult, ALU.add,
                        )
                    # accumulator copy (gpsimd can't read PSUM; keep on DVE,
                    # after the critical tx1 update)
                    acc_new = acc_pool.tile([D, Bs], f32, tag="accp")
                    nc.vector.scalar_tensor_tensor(
                        acc_new[:], ps3[:], float(a_acc[e]), base[:],
                        ALU.mult, ALU.add,
                    )
                    acc_prev = acc_new
                    if not last_eval:
                        ps1 = ps1_next

                th_cur = acc_prev  # theta_{s+1}

            nc.sync.dma_start(d_out[:], th_cur[:])

    # ---- per-core input maps ----
    in_maps = []
    for c in range(N_CORES):
        sl = slice(c * Bs, (c + 1) * Bs)
        th_T = np.ascontiguousarray(np.asarray(theta0[sl], np.float32).T)
        ctx_T = np.ascontiguousarray(np.asarray(context[sl], np.float32).T)
        ctxpack = np.ascontiguousarray(np.concatenate([
            ctx_T.astype(_bf16),
            w1c2_h[:, H : 2 * H].astype(_bf16),
            w1c2_h[:, 0:H].astype(_bf16),
        ], axis=1))
        thpack = np.ascontiguousarray(np.concatenate([
            th_T.astype(_bf16), w1th_h.astype(_bf16)
        ], axis=1))
        in_maps.append(
            {
                "th0": th_T,
                "ctxpack": ctxpack,
                "thpack": thpack,
                "w2": w2_h.astype(_f8np),
                "w3": w3_h.astype(_bf16),
                "biast": bias_t_h,
                **({} if b3_is_zero else {"onesb3": onesb3_h}),
            }
        )

    return nc, in_maps


def _build_and_run(theta0, context, W1, b1, W2, b2, W3, b3, n_steps):
    from concourse.bass_utils import run_bass_kernel_spmd

    nc, in_maps = _build_program(theta0, context, W1, b1, W2, b2, W3, b3, n_steps)
    nc.finalize()  # Bacc: split multi-sem waits + allocate registers
    res = run_bass_kernel_spmd(
        nc,
        in_maps,
        core_ids=list(range(N_CORES)),
        trace=bool(int(os.environ.get("KERNEL_TRACE", "0"))),
    )
    _build_and_run.last_results = res

    out = np.concatenate([r["out"].T for r in res.results], axis=0)
    return np.ascontiguousarray(out.astype(np.float32))


def kernel(theta0, context, W1, b1, W2, b2, W3, b3, n_steps):
    return _build_and_run(
        np.asarray(theta0), np.asarray(context), W1, b1, W2, b2, W3, b3, n_steps
    )


# revision 37
# speedup vs baseline: 1.0219x; 1.0219x over previous
"""CCNF RK4 sampling kernel for 8 Trainium2 NeuronCores.

Data-parallel: batch 2048 -> 256 per core, weights replicated.
On-core layout: features on partitions, batch on the free dim (N=256).
Layer 1/3 matmuls in bf16 (1 cyc/row); layer 2 — the FLOP bulk — in
fp8-e4m3 with DoubleRow perf mode (0.5 cyc/row AND K=256 packed per
matmul: 16 pair-MMs replace 32 bf16 MMs, 4x less PE time). Measured
rel err 8.4e-03 vs the 2e-2 gate (numpy-probed first: fp8 L2 alone
contributes ~8e-3; bf16 everywhere was 9.4e-4).

Over the v1 baseline (463066 -> 440911 ns, cost model):
  - fp8 DoubleRow layer 2: w2 shipped as [128, pair, k-plane, cols],
    h1 GLU outputs written as fp8 pair tiles [128, 2, 256] that are the
    DoubleRow moving operand directly.
  - t-row folded into a host-precomputed per-t bias table (33 distinct
    t values), removing the per-eval memset from the serial RK4 chain.
  - bf16/fp8 weights and activations shrink the weight DMA footprint.
  - startup DMA chain minimized: HWDGE prep is 625ns/DMA (serialized)
    and completion-sem propagation 900ns, so the L1-critical tensors
    ship as two packed DMAs ([ctx | W1-ctx-b | W1-ctx-a] and
    [theta0 | W1-theta]), w2 streams per k-chunk, bias tables ride one
    DMA, and the unused ones/b3 constant is skipped when b3 == 0.
  - L3 shares the 8-bank PSUM ring (no dedicated bank).

Per-eval steady state (cost model): PE 3.0us (43% busy) — the eval is
now latency-bound on the sigmoid->GLU pipeline (DVE 58%, ACT 47%; 8
chunks x ~590ns ACT/DVE cadence plus the RK4 theta-update tail).
Next lever if revisited: pair adjacent PSUM banks into [128,2,2,256]
tiles so sigma/GLU run 4 double-width ops (needs biases moved into
matmuls via a ones-row, using the idle PE). Measured dead ends (all
worse): 128-column stream splits (per-op ACT/DVE overheads exceed the
latency hidden), kc-rotation in L2 groups (removes scheduler freedom),
PSUM pre-fill / filler shuffling (wall = work + exposed chain is
invariant under any reordering the scheduler can already do).
"""

import os

import numpy as np
from ml_dtypes import bfloat16 as _bf16
from ml_dtypes import float8_e4m3 as _f8np

N_CORES = 8


def _build_program(theta0, context, W1, b1, W2, b2, W3, b3, n_steps):
    import concourse.bass as bass
    import concourse.mybir as mybir
    import concourse.tile as tile
    from concourse import bacc

    f32 = mybir.dt.float32
    f32r = mybir.dt.float32r
    bf16 = mybir.dt.bfloat16
    f8 = mybir.dt.float8e4
    DR = mybir.MatmulPerfMode.DoubleRow
    ALU = mybir.AluOpType
    SIGMOID = mybir.ActivationFunctionType.Sigmoid

    B, D = theta0.shape          # 2048, 32
    C = context.shape[1]         # 128
    IN, H2 = W1.shape            # 161, 1024
    H = W2.shape[0]              # 512
    assert H2 == 2 * H and W2.shape[1] == 2 * H and W3.shape == (H, D)
    assert IN == D + 1 + C
    assert B % N_CORES == 0
    Bs = B // N_CORES            # 256 per core
    steps = int(n_steps)
    dt = 1.0 / steps

    KC = H // 128                # 4 k-chunks for layer 2/3
    MJ = H // 128                # 4 output column-chunks per GLU half
    K1B = C                      # 128 ctx rows

    # ---- host-side layout prep (shared across cores) ----
    W1 = np.asarray(W1, np.float32)
    w1th_h = np.ascontiguousarray(W1[:D])                      # [32, 1024] theta rows
    w1t_row = W1[D]                                            # [1024] time row
    w1c2_h = np.ascontiguousarray(W1[D + 1 :])                 # [128, 1024] ctx rows
    b1 = np.asarray(b1, np.float32)
    b2 = np.asarray(b2, np.float32)
    b3_is_zero = not np.any(np.asarray(b3, np.float32))
    # DoubleRow layout: [512,1024] -> [128, pair(2), plane(2), 1024]
    # (pair P covers kc = 2P, 2P+1; plane i is the kc = 2P+i k-tile)
    KCP = KC // 2
    w2_h = np.ascontiguousarray(
        np.asarray(W2, np.float32)
        .reshape(KCP, 2, 128, 2 * H).transpose(2, 0, 1, 3)
        .reshape(128, KCP * 2 * 2 * H)
    )
    # [512,32] -> [128, 4*32]
    w3_h = np.ascontiguousarray(
        np.asarray(W3, np.float32).reshape(KC, 128, D).transpose(1, 0, 2).reshape(128, KC * D)
    )
    # per-t layer-1 bias table: bias(t) = b1 + t * w1t_row, t = k*dt/2 for
    # k = 0..2*steps. layout per t: [a-half j cols | b-half j cols] = [128, 8]
    NT = 2 * steps + 1
    tvals = (np.arange(NT, dtype=np.float32) * (dt / 2.0)).reshape(NT, 1)
    btab = b1.reshape(1, 2 * H) + tvals * w1t_row.reshape(1, 2 * H)  # [NT, 1024]
    btab = btab.reshape(NT, 2, MJ, 128)                              # (t, half, j, p)
    bias_t_h = np.ascontiguousarray(
        btab.transpose(3, 0, 1, 2).reshape(128, NT * 2 * MJ)
    )                                                          # [128, NT*8]
    bias2_h = np.concatenate([
        b2[:H].reshape(MJ, 128).T, b2[H:].reshape(MJ, 128).T,
    ], axis=1)                                                 # [128, 8]
    bias_t_h = np.ascontiguousarray(
        np.concatenate([bias_t_h, bias2_h], axis=1)
    )                                                          # [128, NT*8+8]
    ctxpack_h = None  # built per-core (contains the ctx shard)
    onesb3_h = np.ascontiguousarray(np.concatenate([
        np.ones((1, Bs), np.float32),
        np.asarray(b3, np.float32).reshape(1, D),
    ], axis=1))                                                # [1, Bs+32]
    # onesb3 only ships when b3 is nonzero (it is zero for this problem)

    # ---- build the bass program (same program on all 8 cores) ----
    nc = bacc.Bacc("TRN2", target_bir_lowering=False)

    d_th0 = nc.dram_tensor("th0", [D, Bs], f32r, kind="ExternalInput")
    # packed: [x2 (Bs) | w1c2 b-half (H) | w1c2 a-half (H)] — one partition
    # group, two DMAs (critical prefix first), one HWDGE prep each
    d_ctxpack = nc.dram_tensor("ctxpack", [K1B, Bs + 2 * H], bf16,
                               kind="ExternalInput")
    # packed: [th0h (Bs) | w1th (2H)]
    d_thpack = nc.dram_tensor("thpack", [D, Bs + 2 * H], bf16,
                              kind="ExternalInput")
    d_w2 = nc.dram_tensor("w2", [128, KCP * 2 * 2 * H], f8, kind="ExternalInput")
    d_w3 = nc.dram_tensor("w3", [128, KC * D], bf16, kind="ExternalInput")
    d_biast = nc.dram_tensor("biast", [128, NT * 2 * MJ + 2 * MJ], f32, kind="ExternalInput")
    d_ob3 = (None if b3_is_zero else
             nc.dram_tensor("onesb3", [1, Bs + D], f32r, kind="ExternalInput"))
    d_out = nc.dram_tensor("out", [D, Bs], f32, kind="ExternalOutput")

    # RK4 coefficients: arg scale (for next eval's input), acc scale
    c_arg = [0.5 * dt, 0.5 * dt, dt]
    a_acc = [dt / 6.0, dt / 3.0, dt / 3.0, dt / 6.0]
    # t index per (step, eval): t = (s + TOFF[e]) * dt -> idx = 2s + IOFF[e]
    IOFF = (0, 1, 1, 2)

    FSPLIT = int(os.environ.get("KERNEL_FSPLIT", "2"))  # ctx banks in stall-1 slot

    with tile.TileContext(nc) as tc:
        with (
            tc.tile_pool(name="const", bufs=1) as cpool,
            tc.tile_pool(name="psmm", bufs=8, space="PSUM") as ps_pool,
            tc.tile_pool(name="sig", bufs=int(os.environ.get("KERNEL_SIGB", "10"))) as sig_pool,
            tc.tile_pool(name="hact", bufs=int(os.environ.get("KERNEL_HB", "20"))) as h_pool,
            tc.tile_pool(name="accp", bufs=int(os.environ.get("KERNEL_AB", "6"))) as acc_pool,
        ):
            tctx = cpool.tile([K1B, Bs + 2 * H], bf16)
            tx2 = tctx[:, 0:Bs]
            # w1c2 columns: b-half at [Bs : Bs+H], a-half at [Bs+H : Bs+2H]
            tthp = cpool.tile([D, Bs + 2 * H], bf16)
            tx1 = tthp[:, 0:Bs]
            tw1th = tthp[:, Bs : Bs + 2 * H]
            tw2 = cpool.tile([128, KCP, 2, 2 * H], f8)
            tw3 = cpool.tile([128, KC * D], bf16)
            tbiast = cpool.tile([128, NT * 2 * MJ + 2 * MJ], f32)
            tb2a = tbiast[:, NT * 2 * MJ : NT * 2 * MJ + MJ]
            tb2b = tbiast[:, NT * 2 * MJ + MJ : NT * 2 * MJ + 2 * MJ]
            if not b3_is_zero:
                tob3 = cpool.tile([1, Bs + D], f32r)
                tones = tob3[:, 0:Bs]
                tb3 = tob3[:, Bs : Bs + D]
            tth0 = cpool.tile([D, Bs], f32r)    # initial theta

            def w1c2_col(mj):
                # mj >= MJ: b-half chunk, else a-half chunk
                if mj >= MJ:
                    base = Bs + (mj - MJ) * 128
                else:
                    base = Bs + H + mj * 128
                return tctx[:, base : base + 128]

            def tb1a(idx, j):
                return tbiast[:, idx * 2 * MJ + j : idx * 2 * MJ + j + 1]

            def tb1b(idx, j):
                return tbiast[:, idx * 2 * MJ + MJ + j : idx * 2 * MJ + MJ + j + 1]

            # L1-critical tensors first so eval 0 can start while w2/w3
            # still stream; w2 split per k-chunk so layer 2 can begin
            # before the full weight matrix lands.
            nc.sync.dma_start(tctx[:, 0 : Bs + H], d_ctxpack[:, 0 : Bs + H])
            nc.sync.dma_start(tctx[:, Bs + H : Bs + 2 * H],
                              d_ctxpack[:, Bs + H : Bs + 2 * H])
            nc.sync.dma_start(tthp[:], d_thpack[:])
            nc.sync.dma_start(tbiast[:], d_biast[:])
            for P in range(KCP):
                nc.sync.dma_start(
                    tw2[:, P, :, :],
                    d_w2[:, P * 2 * 2 * H : (P + 1) * 2 * 2 * H],
                )
                if P == 0:
                    nc.sync.dma_start(tth0[:], d_th0[:])
            nc.sync.dma_start(tw3[:], d_w3[:])
            if not b3_is_zero:
                nc.sync.dma_start(tob3[:], d_ob3[:])

            def mm(out_ap, lhsT_ap, rhs_ap, start, stop, pm=None):
                nc.tensor.matmul(out_ap, lhsT_ap, rhs_ap, start=start,
                                 stop=stop, perf_mode=pm)

            th_cur = tth0       # theta at start of current step

            def issue_l1ctx(js):
                # static context contribution for the NEXT eval's layer 1,
                # placed in PE stall windows. One accumulation group per
                # bank; stop goes on the last theta MM next eval.
                tiles = []
                for j in js:
                    ps = ps_pool.tile([128, 2 * Bs], f32, tag="psmm")
                    for half, mj in ((1, j + MJ), (0, j)):
                        dst = ps[:, half * Bs : (half + 1) * Bs]
                        mm(dst, w1c2_col(mj), tx2[:],
                           start=(half == 1), stop=False)
                    tiles.append(ps)
                return tiles

            # only 3 of 4 L1 banks are pre-issued: 8 bank allocs per eval on
            # the 8-slot ring (a 4th pre-issue would shift slot reuse into
            # live banks and stall ~650ns/eval on bank-free waits). Bank j=3
            # gets its ctx MMs inline, right before its theta MMs.
            ps1 = issue_l1ctx(range(MJ - 1))

            for s in range(steps):
                for e in range(4):
                    idx = 2 * s + IOFF[e]
                    last_eval = (s == steps - 1) and (e == 3)

                    # ---- layer 1: theta MMs close the pre-issued banks ----
                    # h1 chunks land in fp8 pair tiles [128, plane(2), Bs]
                    # (plane = kc within the pair) feeding DoubleRow L2 MMs
                    h1p = [h_pool.tile([128, 2, Bs], f8, tag="h1t",
                                       name=f"h1p{P}")
                           for P in range(KCP)]
                    for j in range(MJ):
                        if j < len(ps1):
                            ps = ps1[j]
                            first = False
                        else:
                            ps = ps_pool.tile([128, 2 * Bs], f32, tag="psmm")
                            first = True
                        for half, mj in ((1, j + MJ), (0, j)):
                            dst = ps[:, half * Bs : (half + 1) * Bs]
                            if first:
                                # start=True only on the bank's first MM: a
                                # second start would re-mark the whole bank
                                # pending-zero and wipe the b-half sums
                                mm(dst, w1c2_col(mj), tx2[:],
                                   start=(half == 1), stop=False)
                        for half, mj in ((1, j + MJ), (0, j)):
                            dst = ps[:, half * Bs : (half + 1) * Bs]
                            msl = slice(mj * 128, (mj + 1) * 128)
                            mm(dst, tw1th[:, msl], tx1[:], start=False,
                               stop=(half == 0))
                        sg = sig_pool.tile([128, Bs], f32, tag="sig1")
                        nc.scalar.activation(
                            sg[:], ps[:, Bs : 2 * Bs], SIGMOID,
                            bias=tb1b(idx, j)
                        )
                        nc.vector.scalar_tensor_tensor(
                            h1p[j // 2][:, j % 2, :], ps[:, 0:Bs],
                            tb1a(idx, j), sg[:],
                            ALU.add, ALU.mult,
                        )

                    # first chunk of next-eval ctx MMs fills the h1[0] wait
                    if not last_eval:
                        ps1_next = issue_l1ctx(range(FSPLIT))

                    # ---- layer 2 ----
                    h2 = []
                    for j in range(MJ):
                        ps = ps_pool.tile([128, 2 * Bs], f32, tag="psmm")
                        # b-half group first so the sigmoid overlaps the
                        # a-half matmuls
                        dstb = ps[:, Bs : 2 * Bs]
                        for P in range(KCP):
                            csl = slice((j + MJ) * 128, (j + MJ + 1) * 128)
                            mm(dstb, tw2[:, P, :, csl], h1p[P][:],
                               start=(P == 0), stop=(P == KCP - 1), pm=DR)
                        sg = sig_pool.tile([128, Bs], f32, tag="sig2")
                        nc.scalar.activation(
                            sg[:], dstb, SIGMOID, bias=tb2b[:, j : j + 1]
                        )
                        dsta = ps[:, 0:Bs]
                        for P in range(KCP):
                            csl = slice(j * 128, (j + 1) * 128)
                            mm(dsta, tw2[:, P, :, csl], h1p[P][:],
                               start=(P == 0), stop=(P == KCP - 1), pm=DR)
                        ht = h_pool.tile([128, Bs], bf16, tag="h2t")
                        nc.vector.scalar_tensor_tensor(
                            ht[:], dsta, tb2a[:, j : j + 1], sg[:],
                            ALU.add, ALU.mult,
                        )
                        h2.append(ht)

                    # ---- layer 3: k = h2 @ W3 (+ b3) in PSUM ----
                    ps3full = ps_pool.tile([128, 2 * Bs], f32, tag="psmm")
                    ps3 = ps3full[0:D, 0:Bs]
                    for kc in range(KC):
                        mm(ps3[:], tw3[:, kc * D : (kc + 1) * D], h2[kc][:],
                           start=(kc == 0), stop=(kc == KC - 1 and b3_is_zero))
                    if not b3_is_zero:
                        mm(ps3[:], tb3[:], tones[:], start=False, stop=True)

                    # remaining pre-issued ctx MMs fill the tx1 wait at the
                    # boundary (bank j=3 is issued inline next eval)
                    if not last_eval:
                        ps1_next += issue_l1ctx(range(FSPLIT, MJ - 1))

                    # ---- RK4 bookkeeping ----
                    base = th_cur if e == 0 else acc_prev
                    if e < 3:
                        # next eval's theta arg (critical: feeds L1)
                        nc.vector.scalar_tensor_tensor(
                            tx1[:], ps3[:], float(c_arg[e]), th_cur[:],
                            ALU.mult, ALU.add,
                        )
                    elif s != steps - 1:
                        # theta_{s+1} straight into the matmul input tile
                        nc.vector.scalar_tensor_tensor(
                            tx1[:], ps3[:], float(a_acc[e]), base[:],
                            ALU.mult, ALU.add,
                        )
                    # accumulator copy (gpsimd can't read PSUM; keep on DVE,
                    # after the critical tx1 update)
                    acc_new = acc_pool.tile([D, Bs], f32, tag="accp")
                    nc.vector.scalar_tensor_tensor(
                        acc_new[:], ps3[:], float(a_acc[e]), base[:],
                        ALU.mult, ALU.add,
                    )
                    acc_prev = acc_new
                    if not last_eval:
                        ps1 = ps1_next

                th_cur = acc_prev  # theta_{s+1}

            nc.sync.dma_start(d_out[:], th_cur[:])

    # ---- per-core input maps ----
    in_maps = []
    for c in range(N_CORES):
        sl = slice(c * Bs, (c + 1) * Bs)
        th_T = np.ascontiguousarray(np.asarray(theta0[sl], np.float32).T)
        ctx_T = np.ascontiguousarray(np.asarray(context[sl], np.float32).T)
        ctxpack = np.ascontiguousarray(np.concatenate([
            ctx_T.astype(_bf16),
            w1c2_h[:, H : 2 * H].astype(_bf16),
            w1c2_h[:, 0:H].astype(_bf16),
        ], axis=1))
        thpack = np.ascontiguousarray(np.concatenate([
            th_T.astype(_bf16), w1th_h.astype(_bf16)
        ], axis=1))
        in_maps.append(
            {
                "th0": th_T,
                "ctxpack": ctxpack,
                "thpack": thpack,
                "w2": w2_h.astype(_f8np),
                "w3": w3_h.astype(_bf16),
                "biast": bias_t_h,
                **({} if b3_is_zero else {"onesb3": onesb3_h}),
            }
        )

    return nc, in_maps


def _build_and_run(theta0, context, W1, b1, W2, b2, W3, b3, n_steps):
    from concourse.bass_utils import run_bass_kernel_spmd

    nc, in_maps = _build_program(theta0, context, W1, b1, W2, b2, W3, b3, n_steps)
    nc.finalize()  # Bacc: split multi-sem waits + allocate registers
    res = run_bass_kernel_spmd(
        nc,
        in_maps,
        core_ids=list(range(N_CORES)),
        trace=bool(int(os.environ.get("KERNEL_TRACE", "0"))),
    )
    _build_and_run.last_results = res

    out = np.concatenate([r["out"].T for r in res.results], axis=0)
    return np.ascontiguousarray(out.astype(np.float32))


def kernel(theta0, context, W1, b1, W2, b2, W3, b3, n_steps):
    return _build_and_run(
        np.asarray(theta0), np.asarray(context), W1, b1, W2, b2, W3, b3, n_steps
    )
